# revision 31
# baseline (speedup 1.0000x reference)
"""T5-style causal multi-head attention (B=4, S=2048, E=1024, H=16, D=64)
on 8 NeuronCores. Sharding: core c handles batch c//2 and head half c%2
(8 heads). Host sums the two row-parallel partial output projections per
batch.

Device program (SPMD, identical on all cores; data differs):
  stage 1: PE-transpose x blocks (bf16 identity) -> x^T chunks; f32r
           projections qT,kT [hd,tok] (bf16) and v [tok,hd] (bf16, ones
           column appended for the softmax denominator).
  stage 2: qc-outer / head-pair / k-block loop. s = K^T Q per head pair
           (q-range restricted near the diagonal), ACT exp(x/8) -> bf16 P,
           DVE multiply by exp(bias - bias31) table for the 5 near-diagonal
           offsets (masked entries -> 0; far tiles need no correction since
           all buckets saturate to 31 and the per-head constant cancels in
           softmax). PV matmul with M=65 fuses the denominator row.
           Per-(qc,hp) epilogue: copy O^T out of PSUM, reciprocal of the
           denominator, ones-column broadcast matmul, normalize into oT.
  stage 3 (per qc, interleaved): partial out = O^T.T @ Wo_half -> DRAM.
"""
import sys

sys.path.insert(0, "/opt/trn_rl_repo")

import numpy as np
import ml_dtypes

import concourse.bass as bass
import concourse.mybir as mybir
import concourse.tile as tile
from concourse import bacc
from concourse.bass_utils import run_bass_kernel_spmd
from concourse.masks import make_identity

F32, F32R, BF16 = mybir.dt.float32, mybir.dt.float32r, mybir.dt.bfloat16
AF = mybir.ActivationFunctionType
MULT = mybir.AluOpType.mult

B, S, E, H, D = 4, 2048, 1024, 16, 64
HL = H // 2          # heads per core
HD = HL * D          # 512, per-core head dims
NUM_BUCKETS, MAX_DISTANCE = 32, 128
NT = S // 128        # 16 token blocks
NE = E // 128        # 8 embed chunks
NM = 5               # near-diagonal m offsets: m = mi - 3 in {-3..1}

_NC_CACHE = {}


# ---------------------------------------------------------------- host side

def _np_bucket(distance):
    """Mirror reference._relative_position_bucket for causal (distance>=0),
    float32 arithmetic like jnp."""
    max_exact = NUM_BUCKETS // 2  # 16
    is_small = distance < max_exact
    safe = np.maximum(distance, 1).astype(np.float32)
    log_scale = np.log(safe / np.float32(max_exact)).astype(np.float32) / np.float32(
        np.log(np.float32(MAX_DISTANCE / max_exact))
    )
    large = max_exact + (log_scale * np.float32(NUM_BUCKETS - max_exact)).astype(
        np.int32
    )
    large = np.minimum(large, NUM_BUCKETS - 1)
    return np.where(is_small, distance, large)


def _build_ebias(rb8):
    """rb8 [8, 32] -> [512, 5120] bf16: exp(bias - bias31) factor tables for
    m in {-3..1}, masked (q<k) -> 0. Layout (hp,k) x (m,h2,q). For m >= 2
    every in-tile distance is >= 129 > 112, so bucket == 31 everywhere and
    the factor is exactly 1 (the per-head exp(bias31) constant cancels in
    softmax normalization)."""
    qq = np.arange(512)[None, :]
    kk = np.arange(128)[:, None]
    tiles = []
    for mi in range(NM):
        m = mi - 3
        dd = 128 * m + qq - kk  # [128 k, 512 q]
        bucket = _np_bucket(np.maximum(dd, 0))
        vals = np.exp((rb8[:, bucket] - rb8[:, 31][:, None, None]).astype(np.float32))
        vals = np.where(dd[None] >= 0, vals, 0.0).astype(np.float32)
        tiles.append(vals)
    t = np.stack(tiles, axis=0)  # [5 m, 8 h, 128 k, 512 q]
    t = t.reshape(NM, 4, 2, 128, 512).transpose(1, 3, 0, 2, 4)  # hp,k,m,h2,q
    return np.ascontiguousarray(t).reshape(512, NM * 2 * 512).astype(
        ml_dtypes.bfloat16)


# -------------------------------------------------------------- device side

def _build_nc():
    nc = bacc.Bacc(None, target_bir_lowering=False)
    xq_d = nc.dram_tensor("xq", [S, E], F32, kind="ExternalInput")
    xkv_d = nc.dram_tensor("xkv", [S, E], F32, kind="ExternalInput")
    wq_d = nc.dram_tensor("wq", [E, HD], F32, kind="ExternalInput")
    wk_d = nc.dram_tensor("wk", [E, HD], F32, kind="ExternalInput")
    wv_d = nc.dram_tensor("wv", [E, HD], F32, kind="ExternalInput")
    wo_d = nc.dram_tensor("wo", [HD, E], F32, kind="ExternalInput")
    ebias_d = nc.dram_tensor("ebias", [4 * 128, NM * 2 * 512], BF16,
                             kind="ExternalInput")
    out_d = nc.dram_tensor("out", [S, E], F32, kind="ExternalOutput")

    with tile.TileContext(nc) as tc:
        with (
            tc.tile_pool(name="const", bufs=1) as pconst,
            tc.tile_pool(name="persist", bufs=1) as pper,
        ):
            identf = pconst.tile([128, 128], F32)
            make_identity(nc, identf)
            identr = pconst.tile([128, 128], F32R)
            nc.vector.tensor_copy(identr, identf)
            onesf = pconst.tile([1, 128], F32)
            nc.vector.memset(onesf, 1.0)
            onesr = pconst.tile([1, 128], F32R)
            nc.vector.tensor_copy(onesr, onesf)

            qT = pper.tile([128, 4, S], BF16)         # [pair-dims, hp, tok]
            kT = pper.tile([128, 4, S], BF16)
            vA = pper.tile([128, NT, HL * 65], BF16)  # v + ones col per head
            ebias_sb = pper.tile([128, 4, NM * 2 * 512], BF16)
            wo_sb = pper.tile([128, 4, E], F32R)

            vAr = vA.rearrange("p t (h c) -> p t h c", c=65)
            nc.vector.memset(vAr[:, :, :, 64:65], 1.0)
            eb = ebias_sb.rearrange("p hp (m h q) -> p hp m h q", h=2, q=512)

            # ---------------- stage 1: transposes + projections
            # DMA_ENGINES drains in issue order, so weight/ebias loads are
            # chunked and interleaved between the row loads that feed the PE.
            def transpose_pass(x_dram, prow, pxt, pps, dma_hook):
                """Yields (tp, xT tile [128, NE, 2, 128] f32r) per token pair.
                f32r runs the PE transpose at 1.5 cycles/row."""
                for tp in range(NT // 2):
                    row = prow.tile([128, 2, E], F32R, tag="row")
                    for j in range(2):
                        t = tp * 2 + j
                        nc.sync.dma_start(
                            out=row[:, j, :],
                            in_=x_dram[t * 128:(t + 1) * 128, :]
                            .bitcast(F32R))
                    dma_hook(tp)
                    xT = pxt.tile([128, NE, 2, 128], F32R, tag="xT")
                    for j in range(2):
                        for g in range(2):
                            pt = pps.tile([128, 4, 128], F32R, tag="pt")
                            for e4 in range(4):
                                e = 4 * g + e4
                                nc.tensor.transpose(
                                    pt[:, e4, :],
                                    row[:, j, e * 128:(e + 1) * 128], identr)
                            nc.vector.tensor_copy(
                                xT[:, 4 * g:4 * g + 4, j, :], pt)
                    yield tp, xT

            with (
                tc.tile_pool(name="s1w", bufs=1) as p1w,
                tc.tile_pool(name="s1row", bufs=2) as p1r,
                tc.tile_pool(name="s1xt", bufs=2) as p1x,
                tc.tile_pool(name="psT", bufs=4, space="PSUM") as psT,
                tc.tile_pool(name="psP", bufs=4, space="PSUM") as psP,
            ):
                wq_sb = p1w.tile([128, NE, HD], F32R)
                wk_sb = p1w.tile([128, NE, HD], F32R)
                wv_sb = p1w.tile([128, NE, HD], F32R)

                def dma_w(w_sb, w_dr, e):
                    nc.sync.dma_start(
                        out=w_sb[:, e, :],
                        in_=w_dr[e * 128:(e + 1) * 128, :].bitcast(F32R))

                # chunk schedule keeps pass A's DMA near its PE rate.
                # wq must be fully issued at tp0: tp0's projections read all
                # 8 chunks and deps can only point backwards in program order.
                wq_sched = {0: (0, 8)}
                wk_sched = {2: (0, 2), 3: (2, 4), 4: (4, 6), 5: (6, 7),
                            6: (7, 8)}
                wv_sched = {7: (0, 3)}

                def hook_a(tp):
                    for sched, w_sb, w_dr in ((wq_sched, wq_sb, wq_d),
                                              (wk_sched, wk_sb, wk_d),
                                              (wv_sched, wv_sb, wv_d)):
                        if tp in sched:
                            for e in range(*sched[tp]):
                                dma_w(w_sb, w_dr, e)

                def hook_b(tp):
                    if tp == 0:                       # wv: last 5 chunks
                        for e in range(3, 8):
                            dma_w(wv_sb, wv_d, e)
                    elif tp <= 4:                     # ebias: 1 hp per tp
                        nc.sync.dma_start(
                            out=ebias_sb[:, tp - 1, :],
                            in_=ebias_d[(tp - 1) * 128:tp * 128, :])
                    else:                             # wo: 1,1,2 chunks
                        g0, g1 = {5: (0, 1), 6: (1, 2), 7: (2, 4)}[tp]
                        for g in range(g0, g1):
                            nc.sync.dma_start(
                                out=wo_sb[:, g, :],
                                in_=wo_d[g * 128:(g + 1) * 128, :]
                                .bitcast(F32R))

                # pass A: q projection
                for tp, xT in transpose_pass(xq_d, p1r, p1x, psT, hook_a):
                    for hcp in range(2):
                        qps = psP.tile([128, 2, 256], F32, tag="pj")
                        for h2 in range(2):
                            hc = 2 * hcp + h2
                            for e in range(NE):
                                nc.tensor.matmul(
                                    qps[:, h2, :],
                                    wq_sb[:, e, hc * 128:(hc + 1) * 128],
                                    xT[:, e, :, :],
                                    start=(e == 0), stop=(e == NE - 1))
                        nc.vector.tensor_copy(
                            qT[:, 2 * hcp:2 * hcp + 2,
                               tp * 256:(tp + 1) * 256], qps)

                # pass B: k and v projections (shared xkv transpose)
                for tp, xT in transpose_pass(xkv_d, p1r, p1x, psT, hook_b):
                    for hcp in range(2):
                        kps = psP.tile([128, 2, 256], F32, tag="pj")
                        for h2 in range(2):
                            hc = 2 * hcp + h2
                            for e in range(NE):
                                nc.tensor.matmul(
                                    kps[:, h2, :],
                                    wk_sb[:, e, hc * 128:(hc + 1) * 128],
                                    xT[:, e, :, :],
                                    start=(e == 0), stop=(e == NE - 1))
                        nc.vector.tensor_copy(
                            kT[:, 2 * hcp:2 * hcp + 2,
                               tp * 256:(tp + 1) * 256], kps)
                    for j in range(2):
                        vps = psP.tile([128, HD], F32, tag="pj")
                        for e in range(NE):
                            nc.tensor.matmul(
                                vps, xT[:, e, j, :], wv_sb[:, e, :],
                                start=(e == 0), stop=(e == NE - 1))
                        nc.vector.tensor_copy(
                            vAr[:, tp * 2 + j, :, 0:64],
                            vps.rearrange("p (h c) -> p h c", c=64))

            # ---------------- stages 2+3: attention + output projection
            with (
                tc.tile_pool(name="persist2", bufs=1) as pper2,
                tc.tile_pool(name="s2p", bufs=8) as p2p,
                tc.tile_pool(name="s2o", bufs=2) as p2o,
                tc.tile_pool(name="s2r", bufs=2) as p2r,
                tc.tile_pool(name="s3o", bufs=2) as p3o,
                tc.tile_pool(name="psS", bufs=2, space="PSUM") as psS,
                tc.tile_pool(name="psO", bufs=1, space="PSUM") as psO,
                tc.tile_pool(name="psM", bufs=2, space="PSUM") as psM,
            ):
                oT = pper2.tile([128, 4, S], F32R)
                oev_state = {}

                def make_s3(qc):
                    """Output-projection thunks for this qc's 4 token tiles;
                    drained one-per-two-blocks inside the NEXT qc's loop so
                    the contiguous matmul burst never starves ACT."""
                    units = []
                    for t4 in range(4):
                        t = 4 * qc + t4
                        for ec in range(2):
                            def unit(t=t, ec=ec):
                                if ec == 0:
                                    oev_state[t] = p3o.tile(
                                        [128, E], F32, tag="oev", name="oev")
                                oev = oev_state[t]
                                ops = psM.tile([128, 512], F32, tag="mix")
                                for hp in range(4):
                                    nc.tensor.matmul(
                                        ops,
                                        oT[:, hp, t * 128:(t + 1) * 128],
                                        wo_sb[:, hp,
                                              ec * 512:(ec + 1) * 512],
                                        start=(hp == 0), stop=(hp == 3))
                                nc.vector.tensor_copy(
                                    oev[:, ec * 512:(ec + 1) * 512], ops)
                                if ec == 1:
                                    nc.sync.dma_start(
                                        out=out_d[t * 128:(t + 1) * 128, :],
                                        in_=oev)
                            units.append(unit)
                    return units

                s3q = []
                for qc in range(4):
                    for hp in range(4):
                        o = psO.tile([65, 2, 512], F32, tag="o")
                        nkb = 4 * qc + 4

                        def pv(p_kb):
                            # full width: every matmul in a PSUM accumulation
                            # group must cover the same region on hw. Masked
                            # columns hold exact zeros (ebias multiply; pool
                            # slots are zero-initialized).
                            p_, kb_, q0_ = p_kb
                            for hh in range(2):
                                h = 2 * hp + hh
                                nc.tensor.matmul(
                                    o[:, hh, :],
                                    vA[:, kb_, h * 65:(h + 1) * 65],
                                    p_[:, hh, :],
                                    start=(kb_ == 0), stop=(kb_ == nkb - 1))

                        pend = []
                        for kb in range(nkb):
                            m = 4 * qc - kb
                            q0 = max(0, -128 * m)
                            qsl = slice(qc * 512 + q0, (qc + 1) * 512)
                            s = psS.tile([128, 2, 512], F32, tag="s")
                            for hh in range(2):
                                nc.tensor.matmul(
                                    s[:, hh, q0:],
                                    kT[64 * hh:64 * (hh + 1), hp,
                                       kb * 128:(kb + 1) * 128],
                                    qT[64 * hh:64 * (hh + 1), hp, qsl],
                                    start=True, stop=True)
                            p = p2p.tile([128, 2, 512], BF16, tag="p")
                            if q0:
                                # fully-masked columns: written as exact
                                # zeros so the full-width PV adds nothing
                                nc.vector.memset(p[:, :, :q0], 0.0)
                            nc.scalar.activation(p[:, :, q0:], s[:, :, q0:],
                                                 AF.Exp, scale=0.125)
                            if m <= 1:
                                # masked entries multiply to exactly 0
                                nc.vector.tensor_tensor(
                                    out=p[:, :, q0:], in0=p[:, :, q0:],
                                    in1=eb[:, hp, m + 3, :, q0:], op=MULT)
                            # PV issues two blocks late so the PE never
                            # stalls on a block's exp/mult chain
                            pend.append((p, kb, q0))
                            if len(pend) > 2:
                                pv(pend.pop(0))
                            if s3q and kb % 2 == 1:
                                s3q.pop(0)()
                        for pk in pend:
                            pv(pk)
                        # epilogue: reciprocal straight off PSUM (DVE) in
                        # parallel with the O copy (Pool), then normalize
                        rec = p2r.tile([1, 2, 512], F32R, tag="rec")
                        with nc.allow_low_precision(reason="1/den is exact "
                                                    "enough in f32r"):
                            for hh in range(2):
                                nc.vector.reciprocal(rec[:, hh, :],
                                                     o[64:65, hh, :])
                        o_sb = p2o.tile([64, 2, 512], F32, tag="osb")
                        nc.vector.tensor_copy(o_sb, o[0:64, :, :])
                        for hh in range(2):
                            bc = psM.tile([128, 512], F32, tag="mix")
                            nc.tensor.matmul(bc[0:64, :], onesr[:, 0:64],
                                             rec[:, hh, :],
                                             start=True, stop=True)
                            nc.vector.tensor_tensor(
                                out=oT[64 * hh:64 * (hh + 1), hp,
                                       qc * 512:(qc + 1) * 512],
                                in0=o_sb[:, hh, :], in1=bc[0:64, :], op=MULT)
                    for u in s3q:   # leftovers (shouldn't happen)
                        u()
                    s3q = make_s3(qc)
                for u in s3q:       # qc3's output projection
                    u()

    nc.compile()
    return nc


def _get_nc():
    if "nc" not in _NC_CACHE:
        _NC_CACHE["nc"] = _build_nc()
    return _NC_CACHE["nc"]


def _in_maps(inputs_q, inputs_kv, Wq, Wk, Wv, Wo, rel_bias):
    ebs = [_build_ebias(rel_bias[0:HL]), _build_ebias(rel_bias[HL:])]
    in_maps = []
    for c in range(8):
        b, half = c // 2, c % 2
        sl = slice(half * HD, (half + 1) * HD)
        in_maps.append({
            "xq": np.ascontiguousarray(inputs_q[b]),
            "xkv": np.ascontiguousarray(inputs_kv[b]),
            "wq": np.ascontiguousarray(Wq[:, sl]),
            "wk": np.ascontiguousarray(Wk[:, sl]),
            "wv": np.ascontiguousarray(Wv[:, sl]),
            "wo": np.ascontiguousarray(Wo[sl, :]),
            "ebias": ebs[half],
        })
    return in_maps


def kernel(inputs_q, inputs_kv, mask, Wq, Wk, Wv, Wo, rel_bias):
    inputs_q = np.asarray(inputs_q, dtype=np.float32)
    inputs_kv = np.asarray(inputs_kv, dtype=np.float32)
    Wq = np.asarray(Wq, dtype=np.float32)
    Wk = np.asarray(Wk, dtype=np.float32)
    Wv = np.asarray(Wv, dtype=np.float32)
    Wo = np.asarray(Wo, dtype=np.float32)
    rel_bias = np.asarray(rel_bias, dtype=np.float32)

    nc = _get_nc()
    in_maps = _in_maps(inputs_q, inputs_kv, Wq, Wk, Wv, Wo, rel_bias)
    res = run_bass_kernel_spmd(nc, in_maps, core_ids=list(range(8)))
    out = np.stack(
        [res.results[2 * b]["out"] + res.results[2 * b + 1]["out"]
         for b in range(B)])
    return out.astype(np.float32)
